# revision 23
# baseline (speedup 1.0000x reference)
"""Trainium2 Bass kernel for nn_Encoder (GNN message passing encoder).

Computes, for each node b in a batch:
    h[b]  = concat(mean_j feat[neigh[b, j]], feat[nodes[b]]) @ W.T
    out[b] = relu(layernorm(h[b]))          (torch-style unbiased std, eps on std)
returned as [OUT, B] (transposed).

Strategy (8 NeuronCores, data-parallel over the batch):
  - Gathers use the SWDGE extended instruction dma_gather.  The drain side is
    descriptor-rate bound (~5 ns/descriptor aggregate, ~66 ns fixed per
    descriptor per SDMA engine), so the design minimizes descriptor count and
    bytes per descriptor:
      * neighbor rows are gathered from an fp8e4m3 copy of the table
        (values pre-scaled x16 to stay in the normal range; the 1/(16*16)
        dequant+mean factor is folded into W's neighbor half) -> 256B
        descriptors;
      * self rows are gathered from an fp16 copy (self dominates the output
        so it keeps higher precision) -> 512B descriptors.
  - dma_gather indices are signed int16, so the 100k-row table is laid out as
    4 windows of 32767 rows: A=[0,32767) B=[32767,65534) C=[65534,98301)
    D=[67233,100000).  Rows in [67233,98301) are reachable from BOTH C and D;
    each batch element splits its flexible neighbors between C and D to
    equalize its per-window counts, which shrinks the per-tile max-count
    column padding.  Each window is packed with a leading zero row (local
    idx 0) used for padding slots.
  - Batch elements are sorted by their per-window neighbor-count vector
    (self-window as tiebreak) and dealt to (core, tile, partition) so the
    1024 elements sharing a global tile have near-identical counts; per tile
    each window gets max-count columns, plus one self column per window
    present (window-D self columns, which are mostly padding, are shared
    across the group's tiles via partition-sliced identity matmuls).  One
    dma_gather per (tile-group, window, kind) lands all slots
    at [partition p, column c] in [128, C, 256] SBUF buffers (fp8 for
    neighbors, fp16 for self).  Gathers are spread over the 4 SWDGE queues
    by greedy least-loaded assignment.
  - The PE sums each element's neighbor columns (identity-matmul accumulate
    into PSUM[:, 0:256], fp8) and self columns (PSUM[:, 256:512], fp16),
    transposes the combined [128, 512] activations, and applies W^T (with
    the 1/256 neighbor fold).  LayerNorm+ReLU run on ACT/DVE.
  - The program structure (column counts) is input-dependent but identical
    across cores (per-tile max over the 8 cores); only DRAM contents differ.
"""

import os
import sys

sys.path.insert(0, "/opt/trn_rl_repo")
sys.path.insert(0, "/opt/pypackages")

from contextlib import ExitStack

import numpy as np

import concourse.bass as bass
import concourse.tile as tile
from concourse import bacc, mybir
from concourse.bass_utils import run_bass_kernel_spmd
from concourse.masks import make_identity

# Problem constants (hardcoded; kernel.py must be self-contained).
N_NODES, D, OUT, B, K = 100000, 256, 256, 65536, 16
EPS = 1e-6
NCORES = 8
BLOC = B // NCORES  # 8192 nodes per core
P = 128
NT = BLOC // P  # node-tiles per core (64)

WROWS = 32767  # table rows per index window (int16 max)
NW = 4
WSTARTS = (0, 32767, 65534, 67233)
WENDS = tuple(s + WROWS for s in WSTARTS)  # (32767, 65534, 98301, 100000)
FLEX_LO, FLEX_HI = WSTARTS[3], WENDS[2]  # [67233, 98301) reachable from C and D
NEIGH_SCALE = 16.0  # fp8 table pre-scale; folded into W as 1/(K*NEIGH_SCALE)

GROUP = int(os.environ.get("ENC_GROUP", "4"))  # tiles per gather group
# 64KB descriptor rings let SWDGE generation run further ahead of the SDMA
# drain (measured ~8% faster than 16KB; 128KB crashes the device).
DMASCRATCH = int(os.environ.get("ENC_DMASCRATCH", "65536"))
NQUEUES = int(os.environ.get("ENC_QUEUES", "4"))  # SWDGE queues for gathers
GBUFS = int(os.environ.get("ENC_GBUFS", "2"))  # gather pool buffers
GATHER_ONLY = os.environ.get("ENC_GATHER_ONLY", "0") == "1"  # diagnostic
SKIP_GATHER = os.environ.get("ENC_SKIP_GATHER", "0") == "1"  # diagnostic
# Trailing -1 pads crash the device (num_idxs_reg is baked per-program and
# must match the per-core valid count, which differs across cores).  Off.
NEGPAD = os.environ.get("ENC_NEGPAD", "0") == "1"
NEIGH_F8 = os.environ.get("ENC_NEIGH_F8", "1") == "1"  # fp8 neighbor table


def pack_table(feat_table, dt):
    """[zero, rows(A), zero, rows(B), zero, rows(C), zero, rows(D)] -> [4*32768, D]."""
    parts = []
    for w in range(NW):
        parts.append(np.zeros((1, D), dt))
        parts.append(np.ascontiguousarray(feat_table[WSTARTS[w] : WENDS[w]]).astype(dt))
    return np.ascontiguousarray(np.concatenate(parts, 0))


def _wbase(w):
    """First row of window w inside the packed table."""
    return w * (WROWS + 1)


def _classify(r):
    """Forced window of each row; flex band rows return 2 (C) as default."""
    w = np.zeros(r.shape, np.int8)
    w[(r >= WSTARTS[1]) & (r < WENDS[1])] = 1
    w[(r >= WENDS[1]) & (r < FLEX_HI)] = 2
    w[r >= FLEX_HI] = 3
    return w


def _balanced_counts(neigh):
    """Per-element window choice: flexible C/D neighbors split to equalize
    counts.  Returns (counts [nb, NW], win [nb, K] chosen window)."""
    win = _classify(neigh)
    flex = (neigh >= FLEX_LO) & (neigh < FLEX_HI)
    cC = ((win == 2) & ~flex).sum(1)
    cD = (win == 3).sum(1)
    nflex = flex.sum(1)
    give_d = np.clip((cC + nflex - cD + 1) // 2, 0, nflex)
    # assign the first give_d[i] flexible neighbors of row i to D
    fcum = np.cumsum(flex, axis=1)
    to_d = flex & (fcum <= give_d[:, None])
    win = win.copy()
    win[to_d] = 3
    counts = np.stack([(win == w).sum(1) for w in range(NW)], 1)
    return counts, win


def analyze(nodes, neigh_idx, ncores=NCORES, nt=NT, group=GROUP):
    """Sort the batch, compute the shared per-tile column structure, and pack
    per-core int16 gather-index blobs.

    Returns (struct, idx_blobs, order) where struct drives build_program and
    order maps sorted rank -> original batch index.
    """
    nodes = np.asarray(nodes).astype(np.int64).ravel()
    neigh = np.asarray(neigh_idx).astype(np.int64)
    nb = nodes.shape[0]
    assert nb == ncores * nt * P and neigh.shape == (nb, K)

    counts, win = _balanced_counts(neigh)
    wself = _classify(nodes)  # flex selves default to C
    order = np.lexsort(
        (wself,) + tuple(counts[:, w] for w in range(NW - 1, -1, -1))
    )

    # Joint per-tile C/D re-split: with tiles fixed, re-assign each element's
    # flexible neighbors to minimize max(cC) + max(cD) over the global tile.
    flex = (neigh >= FLEX_LO) & (neigh < FLEX_HI)
    forced = _classify(neigh)
    cCmin_all = ((forced == 2) & ~flex).sum(1)
    cDmin_all = (forced == 3).sum(1)
    f_all = flex.sum(1)
    for t in range(nt if os.environ.get("ENC_JOINTCD", "1") == "1" else 0):
        el = order[t * ncores * P : (t + 1) * ncores * P]
        cCmin, cDmin, f = cCmin_all[el], cDmin_all[el], f_all[el]
        best = None
        for Mc in range(int(cCmin.max()), int((cCmin + f).max()) + 1):
            x = np.minimum(f, Mc - cCmin)
            Md = int((cDmin + f - x).max())
            if best is None or Mc + Md < best[0]:
                best = (Mc + Md, Mc, x)
        _, Mc, x = best
        # realize: first x_e flex of each element -> C, rest -> D
        fcum = np.cumsum(flex[el], axis=1)
        to_c = flex[el] & (fcum <= x[:, None])
        wel = win[el]
        wel[flex[el]] = 3
        wel[to_c] = 2
        win[el] = wel
        counts[el, 2] = cCmin + x
        counts[el, 3] = cDmin + f - x

    # Window-D self sharing: D-selves (forced rows >= 98301) are rare
    # (~2 per core-tile), so instead of one D self column per tile, the
    # group's tiles share D column(s): tile t owns partition range
    # [a_t, a_t+L_t) where L_t = max over cores of its D-self count, and the
    # consume uses a partition-sliced identity matmul.  Elements are permuted
    # within each (core, tile) so D-selves sit exactly in the tile's range.
    dshare = os.environ.get("ENC_DSHARE", "1") == "1"
    dL = np.zeros(nt, np.int32)
    if dshare:
        for t in range(nt):
            for c in range(ncores):
                sl = slice((t * ncores + c) * P, (t * ncores + c + 1) * P)
                dL[t] = max(dL[t], int((wself[order[sl]] == 3).sum()))
        # pack tile ranges into shared columns per group.  PE partition
        # slices must be 32-aligned (tile_position constraint), so each tile
        # gets a 32-aligned quarter; 4 quarters per column.  Tiles whose
        # D-self count exceeds a quarter keep a private (classic) column.
        dmeta = []  # per group: dict(t -> (col, a)); ncols
        for g in range((nt + group - 1) // group):
            tiles = list(range(g * group, min((g + 1) * group, nt)))
            cons, slot = {}, 0
            for t in tiles:
                if not 0 < dL[t] <= 32:
                    continue
                # matmul operand base partitions may only be 0, 32, or 64
                cons[t] = (slot // 3, 32 * (slot % 3))
                slot += 1
            dmeta.append(dict(cons=cons, ncols=(slot + 2) // 3))
        # permute within each (core, tile): D-selves -> positions [a_t, a_t+n)
        for t in range(nt):
            if t not in dmeta[t // group]["cons"]:
                continue
            a = dmeta[t // group]["cons"][t][1]
            for c in range(ncores):
                sl = slice((t * ncores + c) * P, (t * ncores + c + 1) * P)
                el = order[sl]
                m = wself[el] == 3
                n = int(m.sum())
                if not n:
                    continue
                newel = np.empty(P, el.dtype)
                posmask = np.zeros(P, bool)
                posmask[a : a + n] = True
                newel[posmask] = el[m]
                newel[~posmask] = el[~m]
                order[sl] = newel
    else:
        dmeta = None

    # per local tile t: elements are sorted ranks [t*8*128, (t+1)*8*128)
    cnt = np.zeros((nt, NW), np.int32)
    sflag = np.zeros((nt, NW), np.int32)
    for t in range(nt):
        el = order[t * ncores * P : (t + 1) * ncores * P]
        cnt[t] = counts[el].max(0)
        for w in range(NW):
            sflag[t, w] = int((wself[el] == w).any())
    if dshare:
        for t in range(nt):
            if t in dmeta[t // group]["cons"]:
                sflag[t, 3] = 0  # shared per group instead
    colcnt = cnt + sflag  # columns per (tile, window)

    ngroups = (nt + group - 1) // group
    # per group: window blocks; per (tile,window): sub-offset inside the block
    # kind 'n' = neighbor (fp8 table), kind 's' = self (fp16 table); they are
    # separate gathers/buffers, so columns are counted per kind.
    ginfo = []
    ioff = 0  # int16 columns consumed so far in the idx blob
    qload = [0] * NQUEUES
    for g in range(ngroups):
        tiles = list(range(g * group, min((g + 1) * group, nt)))
        noff, soff, wmeta = 0, 0, {}
        for w in range(NW):
            ncols = int(cnt[tiles, w].sum())
            scols = int(sflag[tiles, w].sum())
            m = {}
            if ncols:
                q = min(range(NQUEUES), key=lambda i: qload[i])
                qload[q] += ncols * P
                m["n"] = dict(woff=noff, cols=ncols, ioff=ioff, nidx=ncols * P, queue=q)
                noff += ncols
                ioff += ncols * P // 16
            if scols:
                q = min(range(NQUEUES), key=lambda i: qload[i])
                qload[q] += scols * P
                m["s"] = dict(woff=soff, cols=scols, ioff=ioff, nidx=scols * P, queue=q)
                soff += scols
                ioff += scols * P // 16
            if m:
                wmeta[w] = m
        dcons = {}
        if dshare and dmeta[g]["ncols"]:
            dm = dmeta[g]
            q = min(range(NQUEUES), key=lambda i: qload[i])
            qload[q] += dm["ncols"] * P
            wmeta.setdefault(3, {})["d"] = dict(
                woff=soff, cols=dm["ncols"], ioff=ioff,
                nidx=dm["ncols"] * P, queue=q,
            )
            for t, (col, a) in dm["cons"].items():
                dcons[t] = (soff + col, a, a + int(dL[t]))
            soff += dm["ncols"]
            ioff += dm["ncols"] * P // 16
        ginfo.append(dict(tiles=tiles, gcn=noff, gcs=soff, wmeta=wmeta, dcons=dcons))
    gcnmax = max(gi["gcn"] for gi in ginfo)
    gcsmax = max(gi["gcs"] for gi in ginfo)
    struct = dict(
        nt=nt, ncores=ncores, group=group, ngroups=ngroups,
        cnt=cnt, sflag=sflag, colcnt=colcnt, ginfo=ginfo,
        idxtot=ioff, gcnmax=gcnmax, gcsmax=gcsmax,
    )

    # ---- pack per-core idx blobs --------------------------------------
    locs_all = np.zeros(neigh.shape, np.int64)
    for w in range(NW):
        m = win == w
        locs_all[m] = neigh[m] - WSTARTS[w] + 1
    sloc_all = nodes - np.array(WSTARTS)[wself] + 1
    blobs = []
    for c in range(ncores):
        segs = []
        for gi in ginfo:
            for w, mm in gi["wmeta"].items():
                for kind in ("n", "s", "d"):
                    if kind not in mm:
                        continue
                    m = mm[kind]
                    if kind == "d":
                        # shared D column(s): each tile's D-selves sit at its
                        # partition range (enforced by the permute above)
                        dvals = np.zeros((m["cols"], P), np.int64)
                        for t, (gcol, a, b) in gi["dcons"].items():
                            col = gcol - (m["woff"])
                            el = order[(t * ncores + c) * P : (t * ncores + c + 1) * P]
                            sl = np.where(wself[el] == 3, sloc_all[el], 0)
                            dvals[col] += sl
                        vals = dvals.ravel()
                        assert vals.shape[0] == m["nidx"]
                        seg = vals.reshape(-1, 16).T.astype(np.int16)
                        segs.append(np.tile(seg, (8, 1)))
                        continue
                    rows = []  # each row: one column = 128 partition values
                    for t in gi["tiles"]:
                        el = order[(t * ncores + c) * P : (t * ncores + c + 1) * P]
                        if kind == "n":
                            if not cnt[t, w]:
                                continue
                            mask = win[el] == w
                            key = np.argsort(~mask, axis=1, kind="stable")
                            locs = np.take_along_axis(
                                np.where(mask, locs_all[el], 0), key, 1
                            )[:, : cnt[t, w]]  # [128, cnt]
                            rows.append(locs.T)  # [cnt, 128]
                        else:
                            if not sflag[t, w]:
                                continue
                            sl = np.where(wself[el] == w, sloc_all[el], 0)
                            rows.append(sl[None, :])  # [1, 128]
                    vals = np.concatenate(rows, 0).ravel()  # i = col*128 + p
                    assert vals.shape[0] == m["nidx"]
                    if NEGPAD:
                        # trailing pads -> -1 (Q7 drops them pre-generation)
                        nz = np.nonzero(vals)[0]
                        last = nz[-1] if nz.size else -1
                        vals[last + 1 :] = -1
                    seg = vals.reshape(-1, 16).T.astype(np.int16)  # [16, nidx/16]
                    segs.append(np.tile(seg, (8, 1)))  # [128, nidx/16]
        blob = np.ascontiguousarray(np.concatenate(segs, 1))
        assert blob.shape == (P, ioff)
        blobs.append(blob)
    return struct, blobs, order


def build_program(struct, apply_gamma_beta=False, loop_iters=1):
    """Build the Bass program for one core (SPMD across cores)."""
    f8 = mybir.dt.float8e4 if NEIGH_F8 else mybir.dt.float16
    f16 = mybir.dt.float16
    f32 = mybir.dt.float32
    i16 = mybir.dt.int16
    nt = struct["nt"]

    nc = bacc.Bacc(
        "TRN2",
        target_bir_lowering=False,
        debug=False,
        dynamic_dma_scratch_size=DMASCRATCH,
        num_swdge_queues=NQUEUES,
    )
    n_packed = NW * (WROWS + 1)
    feat8 = nc.declare_dram_parameter("feat8", [n_packed, D], f8, isOutput=False)
    feat16 = nc.declare_dram_parameter("feat16", [n_packed, D], f16, isOutput=False)
    wt = nc.declare_dram_parameter("wt", [2 * D, OUT], f16, isOutput=False)
    idxb = nc.declare_dram_parameter(
        "idxb", [P, struct["idxtot"]], i16, isOutput=False
    )
    if apply_gamma_beta:
        gamma_b = nc.declare_dram_parameter("gamma_b", [P, OUT], f32, isOutput=False)
        beta_b = nc.declare_dram_parameter("beta_b", [P, OUT], f32, isOutput=False)
    out_d = nc.declare_dram_parameter("out", [P * nt, OUT], f32, isOutput=True)

    with tile.TileContext(nc) as tc, ExitStack() as ctx:
        consts = ctx.enter_context(tc.tile_pool(name="consts", bufs=1))
        pool_g8 = ctx.enter_context(tc.tile_pool(name="gth8", bufs=GBUFS))
        pool_g16 = ctx.enter_context(tc.tile_pool(name="gth16", bufs=GBUFS))
        pool_c = ctx.enter_context(tc.tile_pool(name="comb", bufs=3))
        pool_f = ctx.enter_context(tc.tile_pool(name="f32s", bufs=3))
        pool_sm = ctx.enter_context(tc.tile_pool(name="small", bufs=4))
        psum_bufs = int(os.environ.get("ENC_PSUM_BUFS", "3"))
        psum_r_pool = ctx.enter_context(
            tc.tile_pool(name="psumR", bufs=psum_bufs, space="PSUM")
        )
        psum_t_pool = ctx.enter_context(tc.tile_pool(name="psumT", bufs=2, space="PSUM"))
        psum_h_pool = ctx.enter_context(
            tc.tile_pool(name="psumH", bufs=psum_bufs, space="PSUM")
        )

        # --- constants ---
        ident32 = consts.tile([P, P], f32)
        make_identity(nc, ident32[:])
        ident = consts.tile([P, P], f16)
        nc.vector.tensor_copy(ident[:], ident32[:])
        ident8 = consts.tile([P, P], f8)
        nc.vector.tensor_copy(ident8[:], ident32[:])

        wt_sb = consts.tile([P, 4 * OUT], f16)
        for c in range(4):
            nc.sync.dma_start(
                out=wt_sb[:, c * OUT : (c + 1) * OUT],
                in_=wt[c * P : (c + 1) * P, :],
            )
        idx_sb = consts.tile([P, struct["idxtot"]], i16)
        nc.sync.dma_start(out=idx_sb[:], in_=idxb[:])
        if apply_gamma_beta:
            gamma_sb = consts.tile([P, OUT], f32)
            nc.sync.dma_start(out=gamma_sb[:], in_=gamma_b[:])
            beta_sb = consts.tile([P, OUT], f32)
            nc.sync.dma_start(out=beta_sb[:], in_=beta_b[:])

        cnt, sflag = struct["cnt"], struct["sflag"]

        def tile_compute(t, gt8, gt16, gi):
            """Sum neighbor/self columns of tile t, project, layernorm."""
            tiles = gi["tiles"]
            ti = tiles.index(t)
            psum_r = psum_r_pool.tile([P, 2 * D], f32, tag="psum_r")
            ncols_list = []
            scols_list = []  # (col, a, b): partition-sliced self adds
            for w, mm in gi["wmeta"].items():
                if "n" in mm and cnt[t, w]:
                    base = mm["n"]["woff"] + int(cnt[tiles[:ti], w].sum())
                    for k in range(int(cnt[t, w])):
                        ncols_list.append(base + k)
                if "s" in mm and sflag[t, w]:
                    base = mm["s"]["woff"] + int(sflag[tiles[:ti], w].sum())
                    scols_list.append((base, 0, P))
            if t in gi["dcons"]:
                scols_list.append(gi["dcons"][t])
            for i, col in enumerate(ncols_list):
                nc.tensor.matmul(
                    psum_r[:, 0:D],
                    lhsT=ident8[:],
                    rhs=gt8[:, col, :],
                    start=(i == 0),
                    stop=(i == len(ncols_list) - 1),
                )
            for i, (col, a, b) in enumerate(scols_list):
                nc.tensor.matmul(
                    psum_r[:, D : 2 * D],
                    lhsT=ident[a:b, :],
                    rhs=gt16[a:b, col, :],
                    start=(i == 0),
                    stop=(i == len(scols_list) - 1),
                )

            comb = pool_c.tile([P, 2 * D], f16, tag="comb")
            nc.vector.tensor_copy(comb[:], psum_r[:])
            psum_t = psum_t_pool.tile([P, 2 * D], f16, tag="psum_t")
            for c in range(4):
                nc.tensor.transpose(
                    psum_t[:, c * P : (c + 1) * P], comb[:, c * P : (c + 1) * P],
                    ident[:],
                )
            combT = pool_c.tile([P, 2 * D], f16, tag="combT")
            nc.vector.tensor_copy(combT[:], psum_t[:])

            psum_h = psum_h_pool.tile([P, OUT], f32, tag="psum_h")
            for c in range(4):
                nc.tensor.matmul(
                    psum_h[:],
                    lhsT=combT[:, c * P : (c + 1) * P],
                    rhs=wt_sb[:, c * OUT : (c + 1) * OUT],
                    start=(c == 0),
                    stop=(c == 3),
                )

            # --- LayerNorm (torch unbiased std, eps added to std) + ReLU ---
            negsum = pool_sm.tile([P, 1], f32, tag="negsum")
            nc.vector.tensor_reduce(
                negsum[:], psum_h[:], mybir.AxisListType.X, mybir.AluOpType.add,
                negate=True,
            )
            negmean = pool_sm.tile([P, 1], f32, tag="negmean")
            nc.vector.tensor_scalar_mul(negmean[:], negsum[:], 1.0 / OUT)
            xc = pool_f.tile([P, OUT], f32, tag="xc")
            nc.scalar.activation(
                xc[:], psum_h[:], mybir.ActivationFunctionType.Identity,
                bias=negmean[:, 0:1],
            )
            sq = pool_f.tile([P, OUT], f32, tag="sq")
            ss = pool_sm.tile([P, 1], f32, tag="ss")
            nc.scalar.activation(
                sq[:], xc[:], mybir.ActivationFunctionType.Square,
                accum_out=ss[:, 0:1],
            )
            sstd = pool_sm.tile([P, 1], f32, tag="sstd")
            nc.scalar.activation(
                sstd[:], ss[:], mybir.ActivationFunctionType.Sqrt,
                scale=1.0 / (OUT - 1),
            )
            seps = pool_sm.tile([P, 1], f32, tag="seps")
            nc.vector.tensor_scalar_add(seps[:], sstd[:], EPS)
            rstd = pool_sm.tile([P, 1], f32, tag="rstd")
            nc.vector.reciprocal(rstd[:], seps[:])

            y = pool_f.tile([P, OUT], f32, tag="y")
            if apply_gamma_beta:
                xg = pool_f.tile([P, OUT], f32, tag="xg")
                nc.vector.tensor_tensor(
                    xg[:], xc[:], gamma_sb[:], mybir.AluOpType.mult
                )
                xgs = pool_f.tile([P, OUT], f32, tag="xgs")
                nc.scalar.activation(
                    xgs[:], xg[:], mybir.ActivationFunctionType.Copy,
                    scale=rstd[:, 0:1],
                )
                yb = pool_f.tile([P, OUT], f32, tag="yb")
                nc.vector.tensor_tensor(
                    yb[:], xgs[:], beta_sb[:], mybir.AluOpType.add
                )
                nc.vector.tensor_scalar_max(y[:], yb[:], 0.0)
            else:
                nc.scalar.activation(
                    y[:], xc[:], mybir.ActivationFunctionType.Relu,
                    scale=rstd[:, 0:1],
                )

            nc.sync.dma_start(out=out_d[t * P : (t + 1) * P, :], in_=y[:])

        const_gt8 = const_gt16 = None
        if SKIP_GATHER:
            const_gt8 = consts.tile([P, struct["gcnmax"], D], f8)
            nc.vector.memset(const_gt8[:], 0.0)
            const_gt16 = consts.tile([P, max(struct["gcsmax"], 1), D], f16)
            nc.vector.memset(const_gt16[:], 0.0)

        def body():
            for gi in struct["ginfo"]:
                if SKIP_GATHER:
                    gt8, gt16 = const_gt8, const_gt16
                else:
                    gt8 = pool_g8.tile([P, struct["gcnmax"], D], f8, tag="gth8")
                    gt16 = pool_g16.tile(
                        [P, max(struct["gcsmax"], 1), D], f16, tag="gth16"
                    )
                    for w, mm in gi["wmeta"].items():
                        if "n" in mm:
                            m = mm["n"]
                            nc.gpsimd.dma_gather(
                                gt8[:, m["woff"] : m["woff"] + m["cols"], :],
                                feat8[_wbase(w) : _wbase(w) + WROWS + 1, :],
                                idx_sb[:, m["ioff"] : m["ioff"] + m["nidx"] // 16],
                                m["nidx"],
                                m["nidx"],
                                D,
                                single_packet=False,
                                queue_num=m["queue"],
                            )
                        for kind in ("s", "d"):
                            if kind not in mm:
                                continue
                            m = mm[kind]
                            nc.gpsimd.dma_gather(
                                gt16[:, m["woff"] : m["woff"] + m["cols"], :],
                                feat16[_wbase(w) : _wbase(w) + WROWS + 1, :],
                                idx_sb[:, m["ioff"] : m["ioff"] + m["nidx"] // 16],
                                m["nidx"],
                                m["nidx"],
                                D,
                                single_packet=False,
                                queue_num=m["queue"],
                            )
                if GATHER_ONLY:
                    sink = pool_c.tile([P, 1], f16, tag="sink")
                    nc.vector.tensor_copy(sink[:], gt16[:, 0, 0:1])
                else:
                    for t in gi["tiles"]:
                        tile_compute(t, gt8, gt16, gi)

        if GATHER_ONLY:
            zed = consts.tile([P, OUT], f32)
            nc.vector.memset(zed[:], 0.0)
            for t in range(nt):
                nc.sync.dma_start(out=out_d[t * P : (t + 1) * P, :], in_=zed[:])
        if loop_iters > 1:
            with tc.For_i(0, loop_iters, 1):
                body()
        else:
            body()

    nc.finalize()
    return nc


def prepare(feat_table, W, gamma, beta, nodes, neigh_idx):
    """Host-side: analyze indices, build program, pack per-core inputs."""
    feat_table = np.asarray(feat_table, dtype=np.float32)
    W = np.asarray(W, dtype=np.float32)
    gamma = np.asarray(gamma, dtype=np.float32)
    beta = np.asarray(beta, dtype=np.float32)
    nodes = np.asarray(nodes).astype(np.int64)
    neigh_idx = np.asarray(neigh_idx).astype(np.int64)

    struct, blobs, order = analyze(nodes, neigh_idx)

    # combined = [neigh_mean ; self]  ->  W^T rows 0:D get 1/(K*NEIGH_SCALE).
    wt_host = np.ascontiguousarray(W.T).astype(np.float32)
    wt_host[:D] *= 1.0 / (K * NEIGH_SCALE)
    wt_host = wt_host.astype(np.float16)

    trivial_affine = bool(np.all(gamma == 1.0) and np.all(beta == 0.0))
    apply_gb = not trivial_affine

    nc = build_program(struct, apply_gamma_beta=apply_gb)

    if NEIGH_F8:
        f8np = mybir.dt.np(mybir.dt.float8e4)
        feat8_dev = pack_table(feat_table * NEIGH_SCALE, f8np)
    else:
        feat8_dev = pack_table(feat_table * NEIGH_SCALE, np.float16)
    feat16_dev = pack_table(feat_table, np.float16)
    in_maps = []
    for c in range(NCORES):
        m = {
            "feat8": feat8_dev,
            "feat16": feat16_dev,
            "wt": wt_host,
            "idxb": blobs[c],
        }
        if apply_gb:
            m["gamma_b"] = np.ascontiguousarray(
                np.broadcast_to(gamma, (P, OUT))
            ).astype(np.float32)
            m["beta_b"] = np.ascontiguousarray(
                np.broadcast_to(beta, (P, OUT))
            ).astype(np.float32)
        in_maps.append(m)
    return nc, in_maps, order


def assemble(results, order):
    out = np.empty((OUT, B), dtype=np.float32)
    for c in range(NCORES):
        ranks = ((np.arange(NT)[:, None] * NCORES + c) * P
                 + np.arange(P)[None, :]).ravel()
        out[:, order[ranks]] = results[c]["out"].T
    return out


def kernel(feat_table, W, gamma, beta, nodes, neigh_idx):
    nc, in_maps, order = prepare(feat_table, W, gamma, beta, nodes, neigh_idx)
    res = run_bass_kernel_spmd(nc, in_maps, list(range(NCORES)))
    return assemble(res.results, order)
